# revision 7
# baseline (speedup 1.0000x reference)
"""Trainium2 Bass kernel for im2col Conv2d dot-product (v4, bf16 hybrid):
out[b, n] = <enc_x[b, n, :], w_flat> + bias.

Data-parallel over batch: 8 batches per NeuronCore x 8 cores.
Per core x [401408, 49] fp32 (~78.7 MB); HBM roofline ~225 us/core.

Key moves (probe-measured rates per [128, 196*49] tile):
  - SWDGE cast-DMA loads x as bf16 (HBM still reads fp32 -> same 220us
    DMA floor, but halves SBUF traffic/footprint; the Pool-queue trigger
    costs only ~0.8us/tile and GpSimd does no other work -> no POOL-port
    contention with DVE).
  - multiply: DVE bf16 tensor_tensor runs in 2x mode (5.26us vs 10.1
    fp32); ~4.5 tile-equivalents go to the otherwise-idle ScalarE as 49
    strided per-k activation muls (0.72us/col, own SBUF ports).
  - segmented reduce: DVE tensor_reduce (10.1us, the only free-axis
    reducer).  DVE total ~177us, ACT ~175us, both under the DMA floor.
  - ScalarE-tile reduces are emitted 2 tiles late so the in-order DVE
    queue never stalls waiting on ACT's 35us multiply chain.
  - output DMAs ride the (otherwise idle) SP HWDGE ring; bias is added
    on the host (elementwise on the small gathered output).
bf16 rounding of x and w gives rel err ~2e-3, well inside the 2e-2 gate.
"""

from contextlib import ExitStack

import numpy as np

import concourse.bass as bass
import concourse.tile as tile
from concourse import mybir

B = 64
WINDOWS = 50176
K = 49
NCORES = 8
BPC = B // NCORES
NWIN = BPC * WINDOWS
P = 128

WBIG = 196
WSMALL = 49
TBIG = 15
TSMALL = 4
assert TBIG * P * WBIG + TSMALL * P * WSMALL == NWIN

# Tile emission order: (windows-per-partition, mult engine).  Small tiles
# lead (fast pipeline fill) and trail (short drain); the 4 ScalarE-mult
# tiles sit early-middle so their long 49-col chains never gate the tail.
TILES = [(WSMALL, "dve"), (WSMALL, "dve"),
         (WBIG, "dve"), (WBIG, "act"), (WBIG, "dve"), (WBIG, "dve"),
         (WBIG, "act"), (WBIG, "dve"), (WBIG, "dve"), (WBIG, "act"),
         (WBIG, "gp"), (WBIG, "dve"), (WBIG, "act"), (WBIG, "gp"),
         (WBIG, "dve"), (WBIG, "dve"), (WBIG, "dve"),
         (WSMALL, "dve"), (WSMALL, "dve")]
ACT_LAG = 3   # act-tile reduce/out deferred this many tiles
GP_LAG = 3    # gp-tile mult deferred (keeps Pool queue free for triggers)

FP32 = mybir.dt.float32
BF16 = mybir.dt.bfloat16

_NC = None


def _build_nc():
    nc = bass.Bass(trn_type="TRN2", debug=False, num_devices=NCORES)

    x = nc.dram_tensor("x", [NWIN, K], FP32, kind="ExternalInput").ap()
    w = nc.dram_tensor("w", [K], FP32, kind="ExternalInput").ap()
    out = nc.dram_tensor("out", [NWIN], FP32, kind="ExternalOutput").ap()

    mult = mybir.AluOpType.mult
    add = mybir.AluOpType.add

    with tile.TileContext(nc) as tc, ExitStack() as ctx:
        consts = ctx.enter_context(tc.tile_pool(name="consts", bufs=1))
        xpool = ctx.enter_context(tc.tile_pool(name="x", bufs=8))
        opool = ctx.enter_context(tc.tile_pool(name="o", bufs=8))

        wb = consts.tile([P, K], FP32)
        wbb = consts.tile([P, K], BF16)
        wb_ap = wb[:]
        wbb_ap = wbb[:]

        def load_weights():
            # emitted after the first tile's DMA trigger so the input
            # stream starts immediately at t=0
            nc.gpsimd.dma_start(
                out=wb_ap,
                in_=bass.AP(tensor=w.tensor, offset=w.offset,
                            ap=[[0, P]] + list(w.ap)),
            )
            nc.vector.tensor_copy(out=wbb_ap, in_=wb_ap)

        def w_bcast(wn):
            return bass.AP(
                tensor=wbb_ap.tensor, offset=wbb_ap.offset,
                ap=[wbb_ap.ap[0], [0, wn], wbb_ap.ap[1]],
            )

        deferred = []   # [(emit_after_tile_idx, fn), ...]

        def finish_tile(xt_ap, wn, win_base, name):
            xt3d = bass.AP(tensor=xt_ap.tensor, offset=xt_ap.offset,
                           ap=[xt_ap.ap[0], [K, wn], [1, K]])
            acc = opool.tile([P, wn], FP32, tag="acc", name=f"acc{name}")
            nc.vector.tensor_reduce(out=acc[:], in_=xt3d,
                                    axis=mybir.AxisListType.X, op=add)
            dst = bass.AP(
                tensor=out.tensor, offset=out.offset + win_base,
                ap=[[wn, P], [1, wn]],
            )
            nc.sync.dma_start(out=dst, in_=acc[:])

        def do_tile(idx, win_base, wn, mode, name, defer_compute=False):
            fd = wn * K
            xt = xpool.tile([P, fd], BF16, tag="xt", name=f"xt{name}")
            xt_ap = xt[:]
            src = bass.AP(
                tensor=x.tensor,
                offset=x.offset + win_base * K,
                ap=[[fd, P], [1, fd]],
            )
            nc.gpsimd.dma_start(out=xt_ap, in_=src)   # SWDGE cast fp32->bf16
            if defer_compute:
                return lambda: _tile_compute(idx, xt_ap, wn, win_base, mode, name)
            _tile_compute(idx, xt_ap, wn, win_base, mode, name)

        def _tile_compute(idx, xt_ap, wn, win_base, mode, name):
            if mode == "dve":
                xt3d = bass.AP(tensor=xt_ap.tensor, offset=xt_ap.offset,
                               ap=[xt_ap.ap[0], [K, wn], [1, K]])
                nc.vector.tensor_tensor(out=xt3d, in0=xt3d,
                                        in1=w_bcast(wn), op=mult)
                finish_tile(xt_ap, wn, win_base, name)
            elif mode == "gp":
                # GpSimd multiply, emitted GP_LAG tiles late so the in-order
                # Pool queue keeps issuing upcoming DMA triggers first
                def emit_mult(a=xt_ap, b=wn):
                    a3d = bass.AP(tensor=a.tensor, offset=a.offset,
                                  ap=[a.ap[0], [K, b], [1, K]])
                    nc.gpsimd.tensor_tensor(out=a3d, in0=a3d,
                                            in1=w_bcast(b), op=mult)
                deferred.append((idx + GP_LAG, emit_mult))
                deferred.append(
                    (idx + GP_LAG + 2,
                     lambda a=xt_ap, b=wn, c=win_base, d=name: finish_tile(a, b, c, d))
                )
            else:
                for kk in range(K):
                    col = bass.AP(
                        tensor=xt_ap.tensor, offset=xt_ap.offset + kk,
                        ap=[xt_ap.ap[0], [K, wn]],
                    )
                    nc.scalar.activation(
                        out=col, in_=col,
                        func=mybir.ActivationFunctionType.Identity,
                        scale=wb_ap[:, kk:kk + 1],
                    )
                deferred.append(
                    (idx + ACT_LAG,
                     lambda a=xt_ap, b=wn, c=win_base, d=name: finish_tile(a, b, c, d))
                )

        def flush_deferred(now):
            while deferred and deferred[0][0] <= now:
                deferred.pop(0)[1]()

        base = 0
        for t, (wn, mode) in enumerate(TILES):
            if t == 0:
                compute0 = do_tile(t, base, wn, mode, f"t{t}",
                                   defer_compute=True)
                load_weights()
                compute0()
            else:
                do_tile(t, base, wn, mode, f"t{t}")
            flush_deferred(t)
            base += P * wn
        flush_deferred(10**9)
        assert base == NWIN

    return nc


def _split_ctrl_waits(nc, max_waits=1):
    """Work around a walrus codegen limit on this build: instructions accept
    only one sync-wait command. Hoist extra waits onto dedicated no-op
    instructions inserted just before, preserving per-engine order."""
    from concourse import mybir

    for f in nc.m.functions:
        for blk in f.blocks:
            insts = blk.instructions
            i = 0
            while i < len(insts):
                ins = insts[i]
                if (
                    ins.sync_info is not None
                    and len(ins.sync_info.on_wait) > max_waits
                ):
                    waits = list(ins.sync_info.on_wait)
                    keep, extra = waits[:max_waits], waits[max_waits:]
                    ins.sync_info.on_wait = keep
                    for j, wchunk in enumerate(extra):
                        nop = mybir.InstNoOp(
                            name=f"{ins.name}-wsplit{j}",
                            sync_info=mybir.SyncInfo(on_wait=[wchunk], on_update=[]),
                            bass_nofuse=True,
                            engine=ins.engine,
                        )
                        nc.register_instruction(nop, overwrite=True)
                        insts.insert(i, nop)
                        i += 1
                i += 1


def _get_nc():
    global _NC
    if _NC is None:
        _NC = _build_nc()
        _split_ctrl_waits(_NC)
    return _NC


def run(enc_x, weight, bias, trace=False, **spmd_kwargs):
    """Run on 8 NeuronCores; returns (out [B, WINDOWS] fp32, BassKernelResults)."""
    from concourse.bass_utils import run_bass_kernel_spmd

    nc = _get_nc()
    xf = np.ascontiguousarray(np.asarray(enc_x), dtype=np.float32).reshape(
        NCORES, NWIN, K
    )
    wf = np.ascontiguousarray(np.asarray(weight), dtype=np.float32).reshape(K)
    bf = float(np.asarray(bias).reshape(-1)[0])
    in_maps = [{"x": xf[i], "w": wf} for i in range(NCORES)]
    res = run_bass_kernel_spmd(
        nc, in_maps, list(range(NCORES)), trace=trace, **spmd_kwargs
    )
    out = np.stack([res.results[i]["out"] for i in range(NCORES)], axis=0)
    out = out.astype(np.float32) + np.float32(bf)
    return out.reshape(B, WINDOWS), res


def kernel(enc_x, weight, bias, windows_nb=None):
    out, _ = run(enc_x, weight, bias)
    return out
